# revision 17
# baseline (speedup 1.0000x reference)
"""Trainium2 Bass kernel for nn_Attention_6545530159375.

Full prefill attention (rope + GQA causal attention + output proj),
sharded over 8 NeuronCores as DP(batch=2) x TP(head-groups=4).

Per core (batch b, shard j): 8 q heads, 2 kv heads, full sequence.
  P1: q^T = (wq_j x_b^T) with rope          -> DRAM scratch [8,128,S]
  P2: k^T (rope) + v                        -> SBUF resident
  P3: per (qt, h): S^T tiles = k^T.T q^T, exp, (tri-mask), PV + ones-matmul
      denominator broadcast, reciprocal, normalize -> attnT SBUF resident
  P4: out = attnT.T woT^T, streamed to HBM.
All matmuls run as float32r (FP22) with fp32 PSUM accumulation.

Host side: transposes/permutes weights (rope pair-permutation baked into
wq/wk columns), runs the same NEFF on all 8 cores with per-core inputs,
then sums the 4 TP partials per batch.
"""

import sys

if "/opt/trn_rl_repo" not in sys.path:
    sys.path.insert(0, "/opt/trn_rl_repo")

import numpy as np

B, S, D, H, KV, HD = 2, 2048, 4096, 32, 8, 128
TPG = 4                 # tensor-parallel groups (x2 data-parallel = 8 cores)
HL = H // TPG           # 8 q heads per core
KVL = KV // TPG         # 2 kv heads per core
FL = HL * HD            # 1024 local features
NREP = HL // KVL * KVL // KVL  # unused; q head h -> kv head h // (HL // KVL)
QT = 512                # query tile (moving dim)
NQT = S // QT           # 4
NKT = S // 128          # 16 k-chunks
ND = D // QT            # 8 output d-chunks
SCALE = 1.0 / float(np.sqrt(HD))
EXP_BIAS = -20.0        # constant shift inside exp; cancels in softmax

_cache = {}


def _build(causal: bool):
    import concourse.mybir as mybir
    import concourse.tile as tile
    from concourse import bacc

    dt = mybir.dt
    f32 = dt.float32
    f32r = dt.float32r
    AF = mybir.ActivationFunctionType
    ALU = mybir.AluOpType

    nc = bacc.Bacc()
    xT = nc.dram_tensor("xT", [D, S], f32r, kind="ExternalInput")
    wqT = nc.dram_tensor("wqT", [D, FL], f32r, kind="ExternalInput")
    wkT = nc.dram_tensor("wkT", [D, KVL * HD], f32r, kind="ExternalInput")
    wvT = nc.dram_tensor("wvT", [D, KVL * HD], f32r, kind="ExternalInput")
    woT = nc.dram_tensor("woT", [FL, D], f32r, kind="ExternalInput")
    onesC = nc.dram_tensor("onesC", [128, 128], f32r, kind="ExternalInput")
    cosP = nc.dram_tensor("cosP", [128, S], f32, kind="ExternalInput")
    sinP = nc.dram_tensor("sinP", [128, S], f32, kind="ExternalInput")
    if causal:
        tri = nc.dram_tensor("tri", [4, 128, QT], f32, kind="ExternalInput")
    else:
        maskT = nc.dram_tensor("maskT", [S, S], f32, kind="ExternalInput")
    outp = nc.dram_tensor("outp", [S, D], f32, kind="ExternalOutput")

    NDCH = D // 128  # 32 contraction chunks

    with tile.TileContext(nc) as tc:
        with (
            tc.tile_pool(name="const", bufs=1) as constp,
            tc.tile_pool(name="dram", bufs=1, space="DRAM") as dramp,
            tc.tile_pool(name="kv", bufs=1) as kvp,
            tc.tile_pool(name="attn", bufs=1) as attnp,
        ):
            ones = constp.tile([128, 128], f32r)
            nc.sync.dma_start(ones, onesC[:, :])
            biasT = constp.tile([128, 1], f32)
            nc.vector.memset(biasT, EXP_BIAS)
            cos_sb = constp.tile([128, S], f32)
            sin_sb = constp.tile([128, S], f32)
            nc.sync.dma_start(cos_sb, cosP[:, :])
            nc.sync.dma_start(sin_sb, sinP[:, :])
            if causal:
                tri_sb = constp.tile([128, 4, QT], f32)
                for p in range(4):
                    nc.sync.dma_start(tri_sb[:, p, :], tri[p])

            qT_dram = dramp.tile([HL, 128, S], f32)
            kT_sb = [kvp.tile([128, S], f32r, name=f"kT{i}") for i in range(KVL)]
            v_sb = [kvp.tile([128, NKT, 128], f32r, name=f"v{i}") for i in range(KVL)]
            attnT_sb = [attnp.tile([128, S], f32r, name=f"aT{h}") for h in range(HL)]

            def rope(dst, raw, swp, qt, pool):
                """raw=[r|i] rows, swp=[i|r] rows (pre-swapped via DMA).
                dst[0:64]=r*c - i*s ; dst[64:128]=r*s + i*c."""
                c = cos_sb[:, qt * QT:(qt + 1) * QT]
                s = sin_sb[:, qt * QT:(qt + 1) * QT]
                tmp = pool.tile([128, QT], f32, name="ropetmp", tag="ropetmp")
                nc.vector.tensor_tensor(dst[0:64], raw[0:64], c[0:64], ALU.mult)
                nc.vector.tensor_tensor(tmp[0:64], swp[0:64], s[0:64], ALU.mult)
                nc.vector.tensor_tensor(
                    dst[0:64], dst[0:64], tmp[0:64], ALU.subtract)
                nc.vector.tensor_tensor(
                    dst[64:128], swp[64:128], s[64:128], ALU.mult)
                nc.vector.tensor_tensor(
                    tmp[64:128], raw[64:128], c[64:128], ALU.mult)
                nc.vector.tensor_tensor(
                    dst[64:128], dst[64:128], tmp[64:128], ALU.add)

            # ---------------- P1: q projection + rope -> DRAM ----------------
            with (
                tc.tile_pool(name="p1psum", bufs=1, space="PSUM") as pp,
                tc.tile_pool(name="p1x", bufs=3) as xp,
                tc.tile_pool(name="p1w", bufs=3) as wp,
                tc.tile_pool(name="p1rope", bufs=3) as ropep,
            ):
                for qt in range(NQT):
                    qpsum = [pp.tile([128, QT], f32, name=f"qp{h}", tag=f"qp{h}")
                             for h in range(HL)]
                    for d in range(NDCH):
                        xt = xp.tile([128, QT], f32r, tag="x")
                        nc.sync.dma_start(
                            xt, xT[d * 128:(d + 1) * 128, qt * QT:(qt + 1) * QT])
                        wt = wp.tile([128, FL], f32r, tag="w")
                        nc.sync.dma_start(wt, wqT[d * 128:(d + 1) * 128, :])
                        for h in range(HL):
                            nc.tensor.matmul(
                                qpsum[h], wt[:, h * 128:(h + 1) * 128], xt,
                                start=(d == 0), stop=(d == NDCH - 1))
                    for h in range(HL):
                        qs = ropep.tile([128, QT], f32, tag="qrope")
                        nc.scalar.copy(qs, qpsum[h])
                        nc.sync.dma_start(
                            qT_dram[h, :, qt * QT:(qt + 1) * QT], qs)

            # ---------------- P2: k (rope) + v -> SBUF ----------------
            with (
                tc.tile_pool(name="p2psum", bufs=1, space="PSUM") as pp2,
                tc.tile_pool(name="p2x", bufs=3) as xp2,
                tc.tile_pool(name="p2w", bufs=3) as wp2,
                tc.tile_pool(name="p2rope", bufs=3) as ropep,
            ):
                for qt in range(NQT):
                    kpsum = [pp2.tile([128, QT], f32, name=f"kp{i}", tag=f"kp{i}")
                             for i in range(KVL)]
                    vpsum = [pp2.tile([128, KVL * HD], f32, name=f"vp{i}", tag=f"vp{i}")
                             for i in range(4)]
                    for d in range(NDCH):
                        xt = xp2.tile([128, QT], f32r, tag="x2")
                        nc.sync.dma_start(
                            xt, xT[d * 128:(d + 1) * 128, qt * QT:(qt + 1) * QT])
                        wkt = wp2.tile([128, KVL * HD], f32r, tag="wk")
                        nc.sync.dma_start(wkt, wkT[d * 128:(d + 1) * 128, :])
                        wvt = wp2.tile([128, KVL * HD], f32r, tag="wv")
                        nc.sync.dma_start(wvt, wvT[d * 128:(d + 1) * 128, :])
                        for i in range(KVL):
                            nc.tensor.matmul(
                                kpsum[i], wkt[:, i * 128:(i + 1) * 128], xt,
                                start=(d == 0), stop=(d == NDCH - 1))
                        for t4 in range(4):
                            nc.tensor.matmul(
                                vpsum[t4], xt[:, t4 * 128:(t4 + 1) * 128], wvt,
                                start=(d == 0), stop=(d == NDCH - 1))
                    for i in range(KVL):
                        ktmp = ropep.tile([128, QT], f32, tag="ktmp")
                        nc.scalar.copy(ktmp, kpsum[i])
                        kswp = ropep.tile([128, QT], f32, tag="kswp")
                        nc.sync.dma_start(kswp[0:64], ktmp[64:128])
                        nc.sync.dma_start(kswp[64:128], ktmp[0:64])
                        rope(kT_sb[i][:, qt * QT:(qt + 1) * QT],
                             ktmp, kswp, qt, ropep)
                        for t4 in range(4):
                            nc.scalar.copy(
                                v_sb[i][:, qt * 4 + t4, :],
                                vpsum[t4][:, i * 128:(i + 1) * 128])

            # ---------------- P3: attention ----------------
            with (
                tc.tile_pool(name="p3sp", bufs=3, space="PSUM") as spp,
                tc.tile_pool(name="p3o", bufs=2, space="PSUM") as opp,
                tc.tile_pool(name="p3d", bufs=2, space="PSUM") as dpp,
                tc.tile_pool(name="p3q", bufs=3) as qp3,
                tc.tile_pool(name="p3pt", bufs=4) as ptp,
                tc.tile_pool(name="p3rec", bufs=2) as recp,
                tc.tile_pool(name="p3m", bufs=NKT + 1) as mp3,
            ):
                for qt in range(NQT):
                    if not causal:
                        mtiles = []
                        for kt in range(NKT):
                            mt = mp3.tile([128, QT], f32, name="mt", tag="mt")
                            nc.sync.dma_start(
                                mt, maskT[kt * 128:(kt + 1) * 128,
                                          qt * QT:(qt + 1) * QT])
                            mtiles.append(mt)
                    nkt = 4 * (qt + 1) if causal else NKT
                    for h in range(HL):
                        kvh = h // (HL // KVL)
                        qraw = qp3.tile([128, QT], f32, tag="qraw")
                        nc.sync.dma_start(
                            qraw, qT_dram[h, :, qt * QT:(qt + 1) * QT])
                        qswp = qp3.tile([128, QT], f32, tag="qswp")
                        nc.sync.dma_start(
                            qswp[0:64],
                            qT_dram[h, 64:128, qt * QT:(qt + 1) * QT])
                        nc.sync.dma_start(
                            qswp[64:128],
                            qT_dram[h, 0:64, qt * QT:(qt + 1) * QT])
                        qtile = qp3.tile([128, QT], f32r, tag="q")
                        rope(qtile, qraw, qswp, qt, qp3)
                        opsum = opp.tile([128, QT], f32, tag="o")
                        dpsum = dpp.tile([128, QT], f32, tag="d")
                        for kt in range(nkt):
                            sp = spp.tile([128, QT], f32, tag="s")
                            nc.tensor.matmul(
                                sp, kT_sb[kvh][:, kt * 128:(kt + 1) * 128],
                                qtile, start=True, stop=True)
                            if not causal:
                                nc.vector.tensor_tensor(
                                    sp, sp, mtiles[kt], ALU.add)
                            pt = ptp.tile([128, QT], f32r, tag="pt")
                            nc.scalar.activation(
                                pt, sp, AF.Exp, bias=biasT, scale=SCALE)
                            if causal and kt >= qt * 4:
                                nc.vector.tensor_tensor(
                                    pt, pt, tri_sb[:, kt - qt * 4, :], ALU.mult)
                            nc.tensor.matmul(
                                opsum, v_sb[kvh][:, kt, :], pt,
                                start=(kt == 0), stop=(kt == nkt - 1))
                            nc.tensor.matmul(
                                dpsum, ones, pt,
                                start=(kt == 0), stop=(kt == nkt - 1))
                        rec = recp.tile([128, QT], f32, tag="rec")
                        nc.vector.reciprocal(rec, dpsum)
                        nc.vector.tensor_tensor(
                            attnT_sb[h][:, qt * QT:(qt + 1) * QT],
                            opsum, rec, ALU.mult)

            # ---------------- P4: output projection ----------------
            with (
                tc.tile_pool(name="p4psum", bufs=4, space="PSUM") as pp4,
                tc.tile_pool(name="p4w", bufs=2) as wp4,
                tc.tile_pool(name="p4o", bufs=4) as op4,
            ):
                for dd in range(ND):
                    wot = wp4.tile([128, HL, QT], f32r, tag="wo")
                    nc.sync.dma_start(
                        wot,
                        woT[:, dd * QT:(dd + 1) * QT].rearrange(
                            "(fo p) n -> p fo n", p=128))
                    for tcn in range(S // 128):
                        wpsum = pp4.tile([128, QT], f32, tag="wps")
                        for f in range(HL):
                            nc.tensor.matmul(
                                wpsum, attnT_sb[f][:, tcn * 128:(tcn + 1) * 128],
                                wot[:, f, :], start=(f == 0), stop=(f == HL - 1))
                        osb = op4.tile([128, QT], f32, tag="osb")
                        nc.scalar.copy(osb, wpsum)
                        nc.sync.dma_start(
                            outp[tcn * 128:(tcn + 1) * 128,
                                 dd * QT:(dd + 1) * QT], osb)
    nc.finalize()
    return nc


_PERM = np.concatenate([np.arange(0, HD, 2), np.arange(1, HD, 2)])


def _is_causal(mask):
    if mask.shape != (S, S):
        return False
    tril = np.tril(np.ones((S, S), dtype=bool))
    if not np.all(mask[tril] == 0.0):
        return False
    return bool(np.all(mask[~tril] <= -1e8))


def kernel(x, wq, wk, wv, wo, cos, sin, mask, start_pos):
    from concourse import bass_utils

    x = np.asarray(x, np.float32)
    wq = np.asarray(wq, np.float32)
    wk = np.asarray(wk, np.float32)
    wv = np.asarray(wv, np.float32)
    wo = np.asarray(wo, np.float32)
    cos = np.asarray(cos, np.float32)
    sin = np.asarray(sin, np.float32)
    mask = np.asarray(mask, np.float32)

    causal = _is_causal(mask)
    key = causal
    if key not in _cache:
        _cache[key] = _build(causal)
    nc = _cache[key]

    ones_c = np.ones((128, 128), np.float32)
    cosP = np.ascontiguousarray(np.tile(cos.T, (2, 1)))
    sinP = np.ascontiguousarray(np.tile(sin.T, (2, 1)))
    if causal:
        k_idx = np.arange(128)[:, None]
        q_idx = np.arange(QT)[None, :]
        tri = np.stack(
            [(p * 128 + k_idx <= q_idx).astype(np.float32) for p in range(4)])
    else:
        maskT = np.ascontiguousarray(mask.T) * np.float32(np.sqrt(HD))

    in_maps = []
    shard_data = []
    for j in range(TPG):
        wq_j = wq[j * FL:(j + 1) * FL].reshape(HL, HD, D)[:, _PERM, :]
        wqT = np.ascontiguousarray(wq_j.reshape(FL, D).T)
        wk_j = wk[j * KVL * HD:(j + 1) * KVL * HD].reshape(KVL, HD, D)[:, _PERM, :]
        wkT = np.ascontiguousarray(wk_j.reshape(KVL * HD, D).T)
        wvT = np.ascontiguousarray(wv[j * KVL * HD:(j + 1) * KVL * HD].T)
        woT = np.ascontiguousarray(wo[:, j * FL:(j + 1) * FL].T)
        shard_data.append((wqT, wkT, wvT, woT))

    xTs = [np.ascontiguousarray(x[b].T) for b in range(B)]
    for c in range(8):
        b, j = divmod(c, TPG)
        wqT, wkT, wvT, woT = shard_data[j]
        m = {
            "xT": xTs[b], "wqT": wqT, "wkT": wkT, "wvT": wvT, "woT": woT,
            "cosP": cosP, "sinP": sinP, "onesC": ones_c,
        }
        if causal:
            m["tri"] = tri
        else:
            m["maskT"] = maskT
        in_maps.append(m)

    global _last_in_maps
    _last_in_maps = in_maps
    res = bass_utils.run_bass_kernel_spmd(nc, in_maps, core_ids=list(range(8)))
    out = np.zeros((B, S, D), np.float32)
    for c in range(8):
        b = c // TPG
        out[b] += res.results[c]["outp"]
    return out
